# revision 46
# baseline (speedup 1.0000x reference)
"""Trainium2 Bass kernel for nn_AEGNet (B=128, O=1024, I=1024).

Math:  out[b,o] = sum_i d[b,i] * w[b,o,i]
       w = softmax_o( rsqrt( (T[o,i] * sigmoid(d[b,i]))^2 + EPS ) )
       T = (input_x/input_y + linkage_add) * (1 + linkage_mul) - output_x/output_y

Key observations exploited here:
  * T is batch-independent: T2 = T*T (O,I) is computed once per core.
  * error^2 = T2[o,i] * s2[b,i] with s2 = sigmoid(d)^2, so the softmax max
    over o is rsqrt(s2 * min_o T2[o,i] + EPS) -- computed analytically from
    a one-time reduce_min instead of a per-tile reduce_max.
  * rsqrt is banned/inaccurate on the scalar engine; rsqrt(t)=exp(-0.5*ln(t))
    keeps every transcendental in the single natural_log_exp table set
    (sigmoid is likewise computed as 1/(1+exp(-d))), so the ACT engine never
    reloads activation tables.
  * With layout (i on partitions, o on free axis) the softmax reductions are
    free-axis vector ops; the final i-contraction sum_i (d/S)*e is a K=128
    matmul per i-chunk with bf16 inputs, accumulated in PSUM.
  * The fused scale/bias affine inside each activation keeps SBUF traffic
    minimal -- measured on HW, any variant that materializes t = s2*T2+eps
    as a tile (to merge activations wider) loses more to SBUF-bandwidth
    contention than it saves in ACT issue overhead.

Sharding: data-parallel over batch, 16 samples per core on 8 cores; the
(1,O,I) parameters are replicated (transposed on the host -- pure layout).
Measured: ~468 us on silicon, rel err ~2e-3 vs the f32 reference (the bf16
matmul inputs dominate the error; fp32-everywhere measures ~1.2e-4).
"""

import numpy as np

B, O, I = 128, 1024, 1024
NCORES = 8
BL = B // NCORES  # 16 samples per core
P = 128           # SBUF partitions
NK = I // P       # 8 i-chunks
EPS = 1e-7

_CACHE = {}


def _build(bl=BL, nk=NK, o=O):
    """Build the SPMD Bass graph (one core's program)."""
    import types
    from contextlib import ExitStack

    import bass_rust as _bass_rust
    import concourse.bacc as bacc
    import concourse.mybir as mybir
    import concourse.tile as tile
    from concourse.hw_specs import get_activation_tables

    f32 = mybir.dt.float32
    bf16 = mybir.dt.bfloat16
    AF = mybir.ActivationFunctionType
    ALU = mybir.AluOpType
    i_total = nk * P

    nc = bacc.Bacc("TRN2", target_bir_lowering=False, debug=False)
    dT = nc.declare_dram_parameter("dT", [i_total, bl], f32, isOutput=False)
    ixT = nc.declare_dram_parameter("ixT", [i_total, o], f32, isOutput=False)
    iyT = nc.declare_dram_parameter("iyT", [i_total, o], f32, isOutput=False)
    oxT = nc.declare_dram_parameter("oxT", [i_total, o], f32, isOutput=False)
    oyT = nc.declare_dram_parameter("oyT", [i_total, o], f32, isOutput=False)
    laT = nc.declare_dram_parameter("laT", [i_total, o], f32, isOutput=False)
    lmT = nc.declare_dram_parameter("lmT", [i_total, o], f32, isOutput=False)
    out = nc.declare_dram_parameter("out", [bl, o], f32, isOutput=True)

    oh = o // 2  # matmul free-dim halves (<=512 per PSUM bank)

    with tile.TileContext(nc) as tc, ExitStack() as ctx:
        t2t_pool = ctx.enter_context(tc.tile_pool(name="t2t", bufs=1))
        par_pool = ctx.enter_context(tc.tile_pool(name="par", bufs=1))
        par2_pool = ctx.enter_context(tc.tile_pool(name="par2", bufs=2))
        wrk_pool = ctx.enter_context(tc.tile_pool(name="wrk", bufs=2))
        e_pool = ctx.enter_context(tc.tile_pool(name="ep", bufs=8))
        osb_pool = ctx.enter_context(tc.tile_pool(name="osb", bufs=1))
        sml_pool = ctx.enter_context(tc.tile_pool(name="sml", bufs=1))
        it_pool = ctx.enter_context(tc.tile_pool(name="it", bufs=4))
        ps_pool = ctx.enter_context(tc.tile_pool(name="ps", bufs=1, space="PSUM"))

        eps_t = sml_pool.tile([P, 1], f32, tag="eps")
        nc.vector.memset(eps_t[:], EPS)

        # s2 = sigmoid(d)^2 = (1/(1+exp(-d)))^2. Chunk 0's s2 is computed
        # up front (the first main-loop Ln gates on it); the rest are
        # deferred into the per-chunk loop so their DVE smalls don't delay
        # chunk 0's T-chain on the in-order vector queue.
        s2T, dTt = [None] * nk, [None] * nk

        def _s2_block(k):
            rows_k = slice(k * P, (k + 1) * P)
            dt_k = sml_pool.tile([P, bl], f32, tag=f"dt_{k}", name=f"dt_{k}")
            nc.sync.dma_start(dt_k[:], dT[rows_k, :])
            dTt[k] = dt_k
            eN = it_pool.tile([P, bl], f32, tag="eN", name=f"eN_{k}")
            nc.scalar.activation(eN[:], dt_k[:], AF.Exp, scale=-1.0)
            den = it_pool.tile([P, bl], f32, tag="den", name=f"den_{k}")
            nc.vector.tensor_scalar(den[:], eN[:], 1.0, None, ALU.add)
            sg = it_pool.tile([P, bl], f32, tag="sg", name=f"sg_{k}")
            nc.vector.reciprocal(sg[:], den[:])
            s2k = sml_pool.tile([P, bl], f32, tag=f"s2_{k}", name=f"s2_{k}")
            nc.vector.tensor_tensor(s2k[:], sg[:], sg[:], ALU.mult)
            s2T[k] = s2k

        _s2_block(0)

        T2T, negm = [], []
        for k in range(nk):
            rows = slice(k * P, (k + 1) * P)
            if k > 0:
                _s2_block(k)
            def _load(pool, tag, src):
                t = pool.tile([P, o], f32, tag=tag, name=f"{tag}_{k}")
                if k == 0:
                    # split the first chunk's loads so they fan across more
                    # DMA queues and shorten the initial ACT stall
                    hp = P // 2
                    nc.sync.dma_start(t[0:hp, :], src[0 : 0 + hp, :])
                    nc.sync.dma_start(
                        t[hp:P, :], src[hp : P, :]
                    )
                else:
                    nc.sync.dma_start(t[:], src[rows, :])
                return t

            piy = _load(par2_pool, "piy", iyT)
            pix = _load(par2_pool, "pix", ixT)
            pox = _load(par2_pool, "pox", oxT)
            poy = _load(par2_pool, "poy", oyT)
            pla = _load(par_pool, "pla", laT)
            plm = _load(par_pool, "plm", lmT)

            # T = (ix/iy + la)*(1+lm) - ox/oy, computed in-place. Reciprocals
            # via the ~2-ULP Newton variant (plain DVE reciprocal is ~6us per
            # (128,1024) tile); two of the multiplies go to the Pool engine.
            scr = par_pool.tile([P, o], f32, tag="scr")
            nc.vector.reciprocal_approx_accurate(piy[:], piy[:], scr[:])
            nc.gpsimd.tensor_tensor(pix[:], pix[:], piy[:], ALU.mult)
            nc.vector.tensor_tensor(pix[:], pix[:], pla[:], ALU.add)
            nc.vector.scalar_tensor_tensor(
                pix[:], plm[:], 1.0, pix[:], ALU.add, ALU.mult
            )
            nc.vector.reciprocal_approx_accurate(poy[:], poy[:], scr[:])
            nc.gpsimd.tensor_tensor(pox[:], pox[:], poy[:], ALU.mult)
            nc.vector.tensor_tensor(pix[:], pix[:], pox[:], ALU.subtract)
            t2 = t2t_pool.tile([P, o], f32, tag=f"t2_{k}")
            nc.scalar.activation(t2[:], pix[:], AF.Square)
            T2T.append(t2)
            t2m = sml_pool.tile([P, 1], f32, tag=f"t2m_{k}")
            nc.vector.tensor_reduce(t2m[:], t2[:], mybir.AxisListType.X, ALU.min)

            # negm = -exp(-0.5*ln(T2min*s2 + EPS)): the softmax max, through
            # the bit-identical affine+spline path as the main-loop z
            # (in=s2, scale=T2min commutes bit-exactly with the tile path).
            lnm = it_pool.tile([P, bl], f32, tag="lnm", name=f"lnm_{k}")
            nc.scalar.activation(
                lnm[:], s2T[k][:], AF.Ln, bias=eps_t[:], scale=t2m[:]
            )
            mth = it_pool.tile([P, bl], f32, tag="mth", name=f"mth_{k}")
            nc.scalar.activation(mth[:], lnm[:], AF.Exp, scale=-0.5)
            nm = sml_pool.tile([P, bl], f32, tag=f"nm_{k}", name=f"nm_{k}")
            nc.vector.tensor_scalar(nm[:], mth[:], -1.0, None, ALU.mult)
            negm.append(nm)

        # Main loop: groups of nq=4 samples (4 x 2 PSUM banks; matmul output
        # must sit at partition base 0), chunk-major inside the group so each
        # chunk's precompute is consumed as soon as it lands. Per (group,
        # chunk): the 4 samples' ln passes write quarters of one wide tile,
        # one merged exp produces z for all 4, and S/1/S/c batch into one
        # reciprocal + one multiply.
        nq = 4
        out_flat = out[:, :].flatten().rearrange("(p f) -> p f", p=1)
        for b0 in range(0, bl, nq):
            pls = [
                ps_pool.tile([1, oh], f32, tag=f"pl{j}", name=f"pl{j}_{b0}")
                for j in range(nq)
            ]
            phs = [
                ps_pool.tile([1, oh], f32, tag=f"ph{j}", name=f"ph{j}_{b0}")
                for j in range(nq)
            ]
            for k in range(nk):
                S = it_pool.tile([P, nq], f32, tag="S", name=f"S_{b0}_{k}")
                es = []
                lt = wrk_pool.tile([P, nq * o], f32, tag="lt", name=f"lt_{b0}_{k}")
                for j in range(nq):
                    b = b0 + j
                    nc.scalar.activation(
                        lt[:, j * o : (j + 1) * o],
                        T2T[k][:],
                        AF.Ln,
                        bias=eps_t[:],
                        scale=s2T[k][:, b : b + 1],
                    )
                z = wrk_pool.tile([P, nq * o], f32, tag="z", name=f"z_{b0}_{k}")
                nc.scalar.activation(z[:], lt[:], AF.Exp, scale=-0.5)
                # During group 0 the vector engine is busy with the chunk
                # precompute chains; take S from ACT's accumulator there so
                # DVE keeps pace with ACT's chunk consumption.
                act_accum = b0 == 0 and k >= 1
                for j in range(nq):
                    b = b0 + j
                    e = e_pool.tile([P, o], bf16, tag="e", name=f"e_{b}_{k}")
                    if act_accum:
                        nc.scalar.activation(
                            e[:],
                            z[:, j * o : (j + 1) * o],
                            AF.Exp,
                            bias=negm[k][:, b : b + 1],
                            accum_out=S[:, j : j + 1],
                        )
                    else:
                        nc.scalar.activation(
                            e[:],
                            z[:, j * o : (j + 1) * o],
                            AF.Exp,
                            bias=negm[k][:, b : b + 1],
                        )
                        nc.vector.tensor_reduce(
                            S[:, j : j + 1], e[:], mybir.AxisListType.X, ALU.add
                        )
                    es.append(e)
                iS = it_pool.tile([P, nq], f32, tag="iS", name=f"iS_{b0}_{k}")
                nc.vector.reciprocal(iS[:], S[:])
                c = it_pool.tile([P, nq], bf16, tag="c", name=f"c_{b0}_{k}")
                nc.vector.tensor_tensor(
                    c[:], iS[:], dTt[k][:, b0 : b0 + nq], ALU.mult
                )
                for j in range(nq):
                    nc.tensor.matmul(
                        pls[j][:],
                        c[:, j : j + 1],
                        es[j][:, 0:oh],
                        start=(k == 0),
                        stop=(k == nk - 1),
                    )
                    nc.tensor.matmul(
                        phs[j][:],
                        c[:, j : j + 1],
                        es[j][:, oh:o],
                        start=(k == 0),
                        stop=(k == nk - 1),
                    )
            osb = osb_pool.tile([1, nq * o], f32, tag="osb")
            for j in range(nq):
                nc.vector.tensor_copy(osb[:, j * o : j * o + oh], pls[j][:])
                nc.vector.tensor_copy(osb[:, j * o + oh : (j + 1) * o], phs[j][:])
            nc.gpsimd.dma_start(
                out_flat[:, b0 * o : (b0 + nq) * o], osb[:]
            )

    # The stock table-load pass picks the FIRST act_info set containing each
    # function (natural_log for Ln, exp_and_others for Exp), reloading tables
    # between nearly every pair of activations (~150 loads x 1.3us). Every
    # activation here (Ln/Exp) lives in natural_log_exp_and_others, so offer
    # the chooser only that set (positions preserved -> ids stay valid).
    keep = {"natural_log_exp_and_others"}

    def _restricted_act_loads(self):
        tables = [
            (k, (v if k in keep else set()))
            for k, v in get_activation_tables(self.m.arch).items()
        ]
        _bass_rust.insert_act_table_loads(self, tables)

    nc.insert_act_table_loads = types.MethodType(_restricted_act_loads, nc)
    nc.compile()
    return nc


def _get_nc():
    if "nc" not in _CACHE:
        _CACHE["nc"] = _build()
    return _CACHE["nc"]


TRACE = False
LAST = None  # last BassKernelResults, for test harness introspection


def kernel(**inputs):
    from concourse.bass_utils import run_bass_kernel_spmd

    global LAST
    data = np.ascontiguousarray(np.asarray(inputs["data"], dtype=np.float32))
    tr = lambda name: np.ascontiguousarray(
        np.asarray(inputs[name], dtype=np.float32).reshape(O, I).T
    )
    ixT, iyT = tr("input_x"), tr("input_y")
    oxT, oyT = tr("output_x"), tr("output_y")
    laT, lmT = tr("linkage_add"), tr("linkage_mul")

    in_maps = []
    for cid in range(NCORES):
        dTc = np.ascontiguousarray(data[cid * BL : (cid + 1) * BL, :].T)
        in_maps.append(
            {
                "dT": dTc,
                "ixT": ixT,
                "iyT": iyT,
                "oxT": oxT,
                "oyT": oyT,
                "laT": laT,
                "lmT": lmT,
            }
        )

    res = run_bass_kernel_spmd(
        _get_nc(), in_maps, core_ids=list(range(NCORES)), trace=TRACE
    )
    LAST = res
    return np.concatenate([r["out"] for r in res.results], axis=0).astype(np.float32)


# revision 48
# speedup vs baseline: 1.2166x; 1.2166x over previous
"""Trainium2 Bass kernel for nn_AEGNet (B=128, O=1024, I=1024).

Math:  out[b,o] = sum_i d[b,i] * w[b,o,i]
       w = softmax_o( rsqrt( (T[o,i] * sigmoid(d[b,i]))^2 + EPS ) )
       T = (input_x/input_y + linkage_add) * (1 + linkage_mul) - output_x/output_y

Key observations exploited here:
  * T is batch-independent: T2 = T*T (O,I) is computed once per core.
  * error^2 = T2[o,i] * s2[b,i] with s2 = sigmoid(d)^2, so the softmax max
    over o is rsqrt(s2 * min_o T2[o,i] + EPS) -- computed analytically from
    a one-time reduce_min instead of a per-tile reduce_max.
  * rsqrt is banned/inaccurate on the scalar engine; rsqrt(t)=exp(-0.5*ln(t))
    keeps every transcendental in the single natural_log_exp table set
    (sigmoid is likewise computed as 1/(1+exp(-d))), so the ACT engine never
    reloads activation tables.
  * With layout (i on partitions, o on free axis) the softmax reductions are
    free-axis vector ops; the final i-contraction sum_i (d/S)*e is a K=128
    matmul per i-chunk with bf16 inputs, accumulated in PSUM.
  * The fused scale/bias affine inside each activation keeps SBUF traffic
    minimal -- measured on HW, any variant that materializes t = s2*T2+eps
    as a tile (to merge activations wider) loses more to SBUF-bandwidth
    contention than it saves in ACT issue overhead.

Sharding: data-parallel over batch, 16 samples per core on 8 cores; the
(1,O,I) parameters are replicated (transposed on the host -- pure layout).
Measured: ~468 us on silicon, rel err ~2e-3 vs the f32 reference (the bf16
matmul inputs dominate the error; fp32-everywhere measures ~1.2e-4).
"""

import numpy as np

B, O, I = 128, 1024, 1024
NCORES = 8
BL = B // NCORES  # 16 samples per core
P = 128           # SBUF partitions
NK = I // P       # 8 i-chunks
EPS = 1e-7

_CACHE = {}


def _build(bl=BL, nk=NK, o=O):
    """Build the SPMD Bass graph (one core's program)."""
    import types
    from contextlib import ExitStack

    import bass_rust as _bass_rust
    import concourse.bacc as bacc
    import concourse.mybir as mybir
    import concourse.tile as tile
    from concourse.hw_specs import get_activation_tables

    f32 = mybir.dt.float32
    bf16 = mybir.dt.bfloat16
    AF = mybir.ActivationFunctionType
    ALU = mybir.AluOpType
    i_total = nk * P

    nc = bacc.Bacc("TRN2", target_bir_lowering=False, debug=False)
    dT = nc.declare_dram_parameter("dT", [i_total, bl], f32, isOutput=False)
    ixT = nc.declare_dram_parameter("ixT", [i_total, o], f32, isOutput=False)
    iyT = nc.declare_dram_parameter("iyT", [i_total, o], f32, isOutput=False)
    oxT = nc.declare_dram_parameter("oxT", [i_total, o], f32, isOutput=False)
    oyT = nc.declare_dram_parameter("oyT", [i_total, o], f32, isOutput=False)
    laT = nc.declare_dram_parameter("laT", [i_total, o], f32, isOutput=False)
    lmT = nc.declare_dram_parameter("lmT", [i_total, o], f32, isOutput=False)
    out = nc.declare_dram_parameter("out", [bl, o], f32, isOutput=True)

    oh = o // 2  # matmul free-dim halves (<=512 per PSUM bank)

    with tile.TileContext(nc) as tc, ExitStack() as ctx:
        t2t_pool = ctx.enter_context(tc.tile_pool(name="t2t", bufs=1))
        par_pool = ctx.enter_context(tc.tile_pool(name="par", bufs=1))
        par2_pool = ctx.enter_context(tc.tile_pool(name="par2", bufs=2))
        wrk_pool = ctx.enter_context(tc.tile_pool(name="wrk", bufs=2))
        e_pool = ctx.enter_context(tc.tile_pool(name="ep", bufs=10))
        osb_pool = ctx.enter_context(tc.tile_pool(name="osb", bufs=1))
        sml_pool = ctx.enter_context(tc.tile_pool(name="sml", bufs=1))
        it_pool = ctx.enter_context(tc.tile_pool(name="it", bufs=6))
        ps_pool = ctx.enter_context(tc.tile_pool(name="ps", bufs=1, space="PSUM"))

        eps_t = sml_pool.tile([P, 1], f32, tag="eps")
        nc.vector.memset(eps_t[:], EPS)

        # s2 = sigmoid(d)^2 = (1/(1+exp(-d)))^2. Chunk 0's s2 is computed
        # up front (the first main-loop Ln gates on it); the rest are
        # deferred into the per-chunk loop so their DVE smalls don't delay
        # chunk 0's T-chain on the in-order vector queue.
        s2T, dTt = [None] * nk, [None] * nk

        def _s2_block(k):
            rows_k = slice(k * P, (k + 1) * P)
            dt_k = sml_pool.tile([P, bl], f32, tag=f"dt_{k}", name=f"dt_{k}")
            nc.sync.dma_start(dt_k[:], dT[rows_k, :])
            dTt[k] = dt_k
            eN = it_pool.tile([P, bl], f32, tag="eN", name=f"eN_{k}")
            nc.scalar.activation(eN[:], dt_k[:], AF.Exp, scale=-1.0)
            den = it_pool.tile([P, bl], f32, tag="den", name=f"den_{k}")
            nc.vector.tensor_scalar(den[:], eN[:], 1.0, None, ALU.add)
            sg = it_pool.tile([P, bl], f32, tag="sg", name=f"sg_{k}")
            nc.vector.reciprocal(sg[:], den[:])
            s2k = sml_pool.tile([P, bl], f32, tag=f"s2_{k}", name=f"s2_{k}")
            nc.vector.tensor_tensor(s2k[:], sg[:], sg[:], ALU.mult)
            s2T[k] = s2k

        _s2_block(0)

        T2T, negm = [], []
        for k in range(nk):
            rows = slice(k * P, (k + 1) * P)
            if k > 0:
                _s2_block(k)
            def _load(pool, tag, src):
                t = pool.tile([P, o], f32, tag=tag, name=f"{tag}_{k}")
                if k == 0:
                    # split the first chunk's loads so they fan across more
                    # DMA queues and shorten the initial ACT stall
                    hp = P // 2
                    nc.sync.dma_start(t[0:hp, :], src[0 : 0 + hp, :])
                    nc.sync.dma_start(
                        t[hp:P, :], src[hp : P, :]
                    )
                else:
                    nc.sync.dma_start(t[:], src[rows, :])
                return t

            piy = _load(par2_pool, "piy", iyT)
            pix = _load(par2_pool, "pix", ixT)
            pox = _load(par2_pool, "pox", oxT)
            poy = _load(par2_pool, "poy", oyT)
            pla = _load(par_pool, "pla", laT)
            plm = _load(par_pool, "plm", lmT)

            # T = (ix/iy + la)*(1+lm) - ox/oy, computed in-place. Reciprocals
            # via the ~2-ULP Newton variant (plain DVE reciprocal is ~6us per
            # (128,1024) tile); two of the multiplies go to the Pool engine.
            scr = par_pool.tile([P, o], f32, tag="scr")
            nc.vector.reciprocal_approx_accurate(piy[:], piy[:], scr[:])
            nc.gpsimd.tensor_tensor(pix[:], pix[:], piy[:], ALU.mult)
            nc.vector.tensor_tensor(pix[:], pix[:], pla[:], ALU.add)
            nc.vector.scalar_tensor_tensor(
                pix[:], plm[:], 1.0, pix[:], ALU.add, ALU.mult
            )
            nc.vector.reciprocal_approx_accurate(poy[:], poy[:], scr[:])
            nc.gpsimd.tensor_tensor(pox[:], pox[:], poy[:], ALU.mult)
            nc.vector.tensor_tensor(pix[:], pix[:], pox[:], ALU.subtract)
            t2 = t2t_pool.tile([P, o], f32, tag=f"t2_{k}")
            nc.scalar.activation(t2[:], pix[:], AF.Square)
            T2T.append(t2)
            t2m = sml_pool.tile([P, 1], f32, tag=f"t2m_{k}")
            nc.vector.tensor_reduce(t2m[:], t2[:], mybir.AxisListType.X, ALU.min)

            # negm = -exp(-0.5*ln(T2min*s2 + EPS)): the softmax max, through
            # the bit-identical affine+spline path as the main-loop z
            # (in=s2, scale=T2min commutes bit-exactly with the tile path).
            lnm = it_pool.tile([P, bl], f32, tag="lnm", name=f"lnm_{k}")
            nc.scalar.activation(
                lnm[:], s2T[k][:], AF.Ln, bias=eps_t[:], scale=t2m[:]
            )
            mth = it_pool.tile([P, bl], f32, tag="mth", name=f"mth_{k}")
            nc.scalar.activation(mth[:], lnm[:], AF.Exp, scale=-0.5)
            nm = sml_pool.tile([P, bl], f32, tag=f"nm_{k}", name=f"nm_{k}")
            nc.vector.tensor_scalar(nm[:], mth[:], -1.0, None, ALU.mult)
            negm.append(nm)

        # Main loop: groups of nq=4 samples (4 x 2 PSUM banks; matmul output
        # must sit at partition base 0), chunk-major inside the group so each
        # chunk's precompute is consumed as soon as it lands. Per (group,
        # chunk): the 4 samples' ln passes write quarters of one wide tile,
        # one merged exp produces z for all 4, and S/1/S/c batch into one
        # reciprocal + one multiply.
        nq = 4
        out_flat = out[:, :].flatten().rearrange("(p f) -> p f", p=1)
        for b0 in range(0, bl, nq):
            pls = [
                ps_pool.tile([1, oh], f32, tag=f"pl{j}", name=f"pl{j}_{b0}")
                for j in range(nq)
            ]
            phs = [
                ps_pool.tile([1, oh], f32, tag=f"ph{j}", name=f"ph{j}_{b0}")
                for j in range(nq)
            ]
            for k in range(nk):
                S = it_pool.tile([P, nq], f32, tag="S", name=f"S_{b0}_{k}")
                es = []
                lt = wrk_pool.tile([P, nq * o], f32, tag="lt", name=f"lt_{b0}_{k}")
                for j in range(nq):
                    b = b0 + j
                    nc.scalar.activation(
                        lt[:, j * o : (j + 1) * o],
                        T2T[k][:],
                        AF.Ln,
                        bias=eps_t[:],
                        scale=s2T[k][:, b : b + 1],
                    )
                z = wrk_pool.tile([P, nq * o], f32, tag="z", name=f"z_{b0}_{k}")
                nc.scalar.activation(z[:], lt[:], AF.Exp, scale=-0.5)
                for j in range(nq):
                    b = b0 + j
                    e = e_pool.tile([P, o], bf16, tag="e", name=f"e_{b}_{k}")
                    nc.scalar.activation(
                        e[:],
                        z[:, j * o : (j + 1) * o],
                        AF.Exp,
                        bias=negm[k][:, b : b + 1],
                    )
                    nc.vector.tensor_reduce(
                        S[:, j : j + 1], e[:], mybir.AxisListType.X, ALU.add
                    )
                    es.append(e)
                iS = it_pool.tile([P, nq], f32, tag="iS", name=f"iS_{b0}_{k}")
                nc.vector.reciprocal(iS[:], S[:])
                c = it_pool.tile([P, nq], bf16, tag="c", name=f"c_{b0}_{k}")
                nc.vector.tensor_tensor(
                    c[:], iS[:], dTt[k][:, b0 : b0 + nq], ALU.mult
                )
                for j in range(nq):
                    nc.tensor.matmul(
                        pls[j][:],
                        c[:, j : j + 1],
                        es[j][:, 0:oh],
                        start=(k == 0),
                        stop=(k == nk - 1),
                    )
                    nc.tensor.matmul(
                        phs[j][:],
                        c[:, j : j + 1],
                        es[j][:, oh:o],
                        start=(k == 0),
                        stop=(k == nk - 1),
                    )
            osb = osb_pool.tile([1, nq * o], f32, tag="osb")
            for j in range(nq):
                nc.vector.tensor_copy(osb[:, j * o : j * o + oh], pls[j][:])
                nc.vector.tensor_copy(osb[:, j * o + oh : (j + 1) * o], phs[j][:])
            nc.gpsimd.dma_start(
                out_flat[:, b0 * o : (b0 + nq) * o], osb[:]
            )

    # The stock table-load pass picks the FIRST act_info set containing each
    # function (natural_log for Ln, exp_and_others for Exp), reloading tables
    # between nearly every pair of activations (~150 loads x 1.3us). Every
    # activation here (Ln/Exp) lives in natural_log_exp_and_others, so offer
    # the chooser only that set (positions preserved -> ids stay valid).
    keep = {"natural_log_exp_and_others"}

    def _restricted_act_loads(self):
        tables = [
            (k, (v if k in keep else set()))
            for k, v in get_activation_tables(self.m.arch).items()
        ]
        _bass_rust.insert_act_table_loads(self, tables)

    nc.insert_act_table_loads = types.MethodType(_restricted_act_loads, nc)
    nc.compile()
    return nc


def _get_nc():
    if "nc" not in _CACHE:
        _CACHE["nc"] = _build()
    return _CACHE["nc"]


TRACE = False
LAST = None  # last BassKernelResults, for test harness introspection


def kernel(**inputs):
    from concourse.bass_utils import run_bass_kernel_spmd

    global LAST
    data = np.ascontiguousarray(np.asarray(inputs["data"], dtype=np.float32))
    tr = lambda name: np.ascontiguousarray(
        np.asarray(inputs[name], dtype=np.float32).reshape(O, I).T
    )
    ixT, iyT = tr("input_x"), tr("input_y")
    oxT, oyT = tr("output_x"), tr("output_y")
    laT, lmT = tr("linkage_add"), tr("linkage_mul")

    in_maps = []
    for cid in range(NCORES):
        dTc = np.ascontiguousarray(data[cid * BL : (cid + 1) * BL, :].T)
        in_maps.append(
            {
                "dT": dTc,
                "ixT": ixT,
                "iyT": iyT,
                "oxT": oxT,
                "oyT": oyT,
                "laT": laT,
                "lmT": lmT,
            }
        )

    res = run_bass_kernel_spmd(
        _get_nc(), in_maps, core_ids=list(range(NCORES)), trace=TRACE
    )
    LAST = res
    return np.concatenate([r["out"] for r in res.results], axis=0).astype(np.float32)
